# revision 14
# baseline (speedup 1.0000x reference)
"""Dilated self-attention Trainium2 kernel (v2: fp8 DoubleRow + engine-split exp).

Math: 3 dilated-attention branches over x (b=4, n=8192, c=128); every branch
decomposes into independent causal attention problems of identical shape
(m=2048 tokens, d=128):
  branch (w=2048, r=1): 4 segments/batch, (w=4096, r=2): 2, (w=8192, r=4): 1
  -> 7 segments/batch x 4 batches = 28 identical tasks -> 8 cores x 4 slots.

Per task the kernel computes the *unnormalized* attention
  U = (E * causal_mask) @ V @ Wo,   dsum = rowsum(E * causal_mask)
with E = exp(S - 2) (the -2 shift keeps E <= 240 = fp8e4 max; it cancels in
the host combine out[p] = sum_b U_b[p] / sum_b dsum_b[p]).

v2 design (per segment, all orientations transposed: keys/c on partitions):
  - XT bf16 [c,2048] from host; Wq (pre-scaled by A/sqrt(c)), Wk, W2=Wv@Wo bf16.
  - QT/KT: bf16 matmuls -> PSUM f32 -> SBUF f32r copies (ACT/DVE balanced).
  - V' = X W2 -> PSUM -> fp8e4 SBUF tiles [128, NT, 128] (DVE convert).
  - scores ST_j = KT_j^T QT_cch in f32r (full PE rate at free 512).
  - exp: per key-tile, split across two engines by a greedy ns-balancer:
      ACT: activation(Exp, scale=1/A, bias=-2) -> fp8e4 directly
      DVE: bit-trick: byte = round(sA + B) via max(sA+B+K, K), K=1.5*2^23 --
           the f32 mantissa low byte IS the fp8e4 encoding of ~exp(s-2)
           (Schraudolph in fp8 bit space; ~3.3% rms/weight, averages out).
  - E tiles pair up [128, 2, 512]; causal masks multiply in-place on GPSIMD.
  - PV and dsum run as fp8 DoubleRow pair-matmuls (2 key-tiles per pass):
      U^T += Vpair^T Epair   [c, 512]   (PSUM accum over pairs)
      dsum += onespair^T Epair  written at psum partition 32*cch (one bank
      holds all 4 chunks; single drain copy per segment).
All segment outputs: u [4,128,2048] f32 (U^T) and d [4,2048] f32.
"""

import sys

if "/opt/trn_rl_repo" not in sys.path:
    sys.path.insert(0, "/opt/trn_rl_repo")

import numpy as np
import ml_dtypes

B, N, C = 4, 8192, 128
M = 2048                 # tokens per segment (same for every branch)
BRANCHES = [(2048, 1), (4096, 2), (8192, 4)]   # (w, r)
N_CORES = 8
SEGS_PER_CORE = 4        # 28 real segments + 4 duplicates
NT = M // 128            # 16 key/token tiles per segment
NCHUNK = M // 512        # 4 query chunks per segment
SCALE = 1.0 / np.sqrt(C)

SHIFT = -2.0                      # exp(s + SHIFT): keeps E <= e^5.3 < 240
EXP_A = 8.0 / np.log(2.0)         # 11.5416 - fp8 bit-space log base
EXP_B = 8.0 * (7.0 + SHIFT / np.log(2.0)) - 0.370   # calibrated offset
EXP_K = 12582912.0                # 1.5 * 2^23 mantissa-alignment constant

# per-element engine cost (ns) + per-instruction overhead, for the balancer
ACT_NS, ACT_OVH = 0.8333, 185.0
DVE_NS, DVE_OVH = 1.0417, 130.0

_NC_CACHE = {}


def _segment_list():
    segs = []
    for b in range(B):
        for (w, r) in BRANCHES:
            for t in range(N // w):
                segs.append((b, w, r, t))
    return segs


class _Balancer:
    def __init__(self):
        self.t = {"act": 0.0, "dve": 0.0}

    def pick(self, width):
        """Pick engine for a job of `width` free elements; returns name."""
        ca = self.t["act"] + width * ACT_NS + ACT_OVH
        cd = self.t["dve"] + width * DVE_NS + DVE_OVH
        if ca <= cd:
            self.t["act"] = ca
            return "act"
        self.t["dve"] = cd
        return "dve"

    def charge(self, eng, width):
        self.t[eng] += width * (ACT_NS if eng == "act" else DVE_NS) + (
            ACT_OVH if eng == "act" else DVE_OVH
        )


def _build_nc(loop_r=None):
    import contextlib

    import concourse.bass as bass
    import concourse.mybir as mybir
    import concourse.tile as tile
    from concourse import bacc
    from concourse.bass import ts
    from concourse.alu_op_type import AluOpType

    f32 = mybir.dt.float32
    f32r = mybir.dt.float32r
    bf16 = mybir.dt.bfloat16
    f8 = mybir.dt.float8e4
    S = SEGS_PER_CORE
    DR = mybir.MatmulPerfMode.DoubleRow
    Exp = mybir.ActivationFunctionType.Exp

    nc = bacc.Bacc(None, target_bir_lowering=False)
    x_in = nc.dram_tensor("xseg", [S, C, M], bf16, kind="ExternalInput")
    wq_in = nc.dram_tensor("wq", [C, C], bf16, kind="ExternalInput")
    wk_in = nc.dram_tensor("wk", [C, C], bf16, kind="ExternalInput")
    w2_in = nc.dram_tensor("w2", [C, C], bf16, kind="ExternalInput")
    mska_in = nc.dram_tensor("mska", [128, 2, 512], f8, kind="ExternalInput")
    mskb_in = nc.dram_tensor("mskb", [128, 2, 256], f8, kind="ExternalInput")
    mska32_in = nc.dram_tensor("mska32", [128, 2, 512], f32, kind="ExternalInput")
    mskb32_in = nc.dram_tensor("mskb32", [128, 2, 256], f32, kind="ExternalInput")
    ones_in = nc.dram_tensor("ones8", [C, 2, 32], f8, kind="ExternalInput")
    u_out = nc.dram_tensor("u", [S, C, M], f32, kind="ExternalOutput")
    d_out = nc.dram_tensor("d", [S, M], f32, kind="ExternalOutput")

    with tile.TileContext(nc) as tc:
        with (
            tc.tile_pool(name="const", bufs=1) as const,
            tc.tile_pool(name="xt", bufs=2) as xt_pool,
            tc.tile_pool(name="qt", bufs=2) as qt_pool,
            tc.tile_pool(name="kt", bufs=2) as kt_pool,
            tc.tile_pool(name="v8", bufs=2) as v_pool,
            tc.tile_pool(name="v8lo", bufs=2) as vlo_pool,
            tc.tile_pool(name="ut", bufs=3) as ut_pool,
            tc.tile_pool(name="dd", bufs=2) as d_pool,
            tc.tile_pool(name="e8", bufs=6) as e8_pool,
            tc.tile_pool(name="e32", bufs=6) as e32_pool,
            tc.tile_pool(name="ps", bufs=2, space="PSUM") as ps_pool,      # 2 banks each
            tc.tile_pool(name="ps_u", bufs=2, space="PSUM") as ps_u_pool,  # 1 bank each
            tc.tile_pool(name="ps_d", bufs=2, space="PSUM") as ps_d_pool,  # 1 bank each
        ):
            wq_sb = const.tile([C, C], bf16)
            wk_sb = const.tile([C, C], bf16)
            w2_sb = const.tile([C, C], bf16)
            nc.sync.dma_start(wq_sb[:], wq_in[:])
            nc.sync.dma_start(wk_sb[:], wk_in[:])
            nc.sync.dma_start(w2_sb[:], w2_in[:])
            mska_sb = const.tile([128, 2, 512], f8)
            mskb_sb = const.tile([128, 2, 256], f8)
            mska32_sb = const.tile([128, 2, 512], f32)
            mskb32_sb = const.tile([128, 2, 256], f32)
            nc.sync.dma_start(mska_sb[:], mska_in[:])
            nc.sync.dma_start(mskb_sb[:], mskb_in[:])
            nc.sync.dma_start(mska32_sb[:], mska32_in[:])
            nc.sync.dma_start(mskb32_sb[:], mskb32_in[:])
            ones_sb = const.tile([C, 2, 32], f8)
            nc.sync.dma_start(ones_sb[:], ones_in[:])
            bias_sb = const.tile([128, 1], mybir.dt.float32)
            nc.vector.memset(bias_sb[:], float(SHIFT))

            loop_cm = (
                tc.For_i(0, loop_r, 1) if loop_r else contextlib.nullcontext()
            )
            with loop_cm:
              for s in range(S):
                bal = _Balancer()

                def copy(eng, dst, src):
                    if eng == "act":
                        nc.scalar.copy(out=dst, in_=src)
                    else:
                        nc.vector.tensor_copy(dst, src)

                # ---- stage 0: X^T bf16 from DRAM
                xt = xt_pool.tile([C, M], bf16)
                nc.sync.dma_start(xt[:], x_in[s])

                # ---- stage 1: projections (Wq pre-scaled by EXP_A/sqrt(c))
                qt = qt_pool.tile([C, M], bf16)
                kt = kt_pool.tile([C, M], bf16)
                for i in range(NCHUNK):
                    p = ps_pool.tile([128, 1024], f32, tag="ps")
                    nc.tensor.matmul(p[:, 0:512], wq_sb[:], xt[:, ts(i, 512)])
                    nc.tensor.matmul(p[:, 512:1024], wk_sb[:], xt[:, ts(i, 512)])
                    e1 = bal.pick(512)
                    copy(e1, qt[:, ts(i, 512)], p[:, 0:512])
                    e2 = bal.pick(512)
                    copy(e2, kt[:, ts(i, 512)], p[:, 512:1024])
                v8 = v_pool.tile([128, NT, C], f8)
                v8lo = vlo_pool.tile([128, NT, C], f8)
                for g in range(NT // 4):
                    pv = ps_pool.tile([128, 1024], f32, tag="ps")
                    for t4 in range(4):
                        nc.tensor.matmul(
                            pv[:, ts(t4, 128)],
                            xt[:, ts(4 * g + t4, 128)],
                            w2_sb[:],
                        )
                    ev = bal.pick(512)
                    v8_sl = v8[:, 4 * g: 4 * g + 4, :].rearrange("p t c -> p (t c)")
                    copy(ev, v8_sl, pv[:, 0:512])
                    # residual: v8lo = fp8(pv - v8); two-term fp8 V keeps PV
                    # accurate to ~0.2% at DoubleRow cost
                    bal.charge("dve", 512)
                    nc.vector.scalar_tensor_tensor(
                        out=v8lo[:, 4 * g: 4 * g + 4, :].rearrange("p t c -> p (t c)"),
                        in0=pv[:, 0:512],
                        scalar=1.0,
                        in1=v8_sl,
                        op0=AluOpType.mult,
                        op1=AluOpType.subtract,
                    )

                # ---- stage 2: attention per 512-query chunk, key tiles in
                # DoubleRow pairs.  Diagonal pairs first (their masks overlap
                # the full pairs' matmuls).
                d_sb = d_pool.tile([1, M], f32)
                for cch in range(NCHUNK):
                    ps_d = ps_d_pool.tile([128, 512], f32)
                    ps_u = ps_u_pool.tile([128, 512], f32)
                    # pair list: (first tile j0, lo, engine, mask8, mask32)
                    pairs = [
                        (4 * cch, 0, "act", mska_sb, None),
                        (4 * cch + 2, 256, "dve", None, mskb32_sb),
                    ]
                    for p2 in range(cch * 2):
                        pairs.append((2 * p2, 0, None, None, None))
                    npairs = len(pairs)
                    for pi, (j0, lo, eng, m8, m32) in enumerate(pairs):
                        w = 512 - lo
                        if eng is None:
                            eng = bal.pick(2 * w)
                        else:
                            bal.charge(eng, 2 * w)
                        psc = ps_pool.tile([128, 1024], f32, tag="ps")
                        for t in (0, 1):
                            nc.tensor.matmul(
                                psc[:, t * 512 + lo: t * 512 + 512],
                                kt[:, ts(j0 + t, 128)],
                                qt[:, cch * 512 + lo: (cch + 1) * 512],
                            )
                        if eng == "act":
                            e8 = e8_pool.tile([128, 2, 512], f8)
                            for t in (0, 1):
                                nc.scalar.activation(
                                    out=e8[:, t, lo:512],
                                    in_=psc[:, t * 512 + lo: t * 512 + 512],
                                    func=Exp,
                                    bias=bias_sb[:],
                                    scale=float(1.0 / EXP_A),
                                )
                            if m8 is not None:
                                nc.gpsimd.tensor_mul(
                                    out=e8[:, :, lo:512],
                                    in0=e8[:, :, lo:512],
                                    in1=m8[:, :, 0:w],
                                )
                            mov = e8[:, :, lo:512]
                        else:
                            e32 = e32_pool.tile([128, 2, 512], f32)
                            for t in (0, 1):
                                nc.vector.tensor_scalar(
                                    out=e32[:, t, lo:512],
                                    in0=psc[:, t * 512 + lo: t * 512 + 512],
                                    scalar1=float(EXP_B + EXP_K),
                                    scalar2=float(EXP_K),
                                    op0=AluOpType.add,
                                    op1=AluOpType.max,
                                )
                            if m32 is not None:
                                nc.gpsimd.tensor_mul(
                                    out=e32[:, :, lo:512],
                                    in0=e32[:, :, lo:512],
                                    in1=m32[:, :, 0:w],
                                )
                            mov = e32[:].bitcast(f8)[:, :, 4 * lo:2048:4]
                        first, last = (pi == 0), (pi == npairs - 1)
                        nc.tensor.matmul(
                            ps_u[:, lo:512], v8[:, j0:j0 + 2, :], mov,
                            start=first, stop=False, perf_mode=DR,
                        )
                        nc.tensor.matmul(
                            ps_u[:, lo:512], v8lo[:, j0:j0 + 2, :], mov,
                            start=False, stop=last, perf_mode=DR,
                        )
                        nc.tensor.matmul(
                            ps_d[0:32, lo:512], ones_sb[:], mov,
                            start=first, stop=last, perf_mode=DR,
                        )
                    ut = ut_pool.tile([128, 512], f32)
                    eu = bal.pick(512)
                    copy(eu, ut[:], ps_u[:])
                    nc.sync.dma_start(u_out[s][:, ts(cch, 512)], ut[:])
                    ed = bal.pick(512)
                    copy(ed, d_sb[:, ts(cch, 512)], ps_d[0:1, :])
                nc.sync.dma_start(d_out[s: s + 1, :], d_sb[:])

    nc.compile()
    return nc


def get_nc(loop_r=None):
    key = ("nc", loop_r)
    if key not in _NC_CACHE:
        _NC_CACHE[key] = _build_nc(loop_r)
    return _NC_CACHE[key]


def _masks():
    """Pair masks. tri[k, q] = 1 iff key k <= query q (within a 128 block).

    maskA (pair of key-tiles t0,t1 at the diagonal, cols 0..511 of the chunk):
      t0: block 0 = tri, blocks 1..3 = 1
      t1: block 0 = 0, block 1 = tri, blocks 2..3 = 1
    maskB (key-tiles t2,t3, cols 256..511):
      t2: block 2 = tri, block 3 = 1;  t3: block 2 = 0, block 3 = tri
    """
    kk = np.arange(128)[:, None]
    qq = np.arange(128)[None, :]
    tri = (kk <= qq).astype(np.float32)
    one = np.ones((128, 128), np.float32)
    zero = np.zeros((128, 128), np.float32)
    mska = np.empty((128, 2, 512), np.float32)
    mska[:, 0, :] = np.concatenate([tri, one, one, one], axis=1)
    mska[:, 1, :] = np.concatenate([zero, tri, one, one], axis=1)
    mskb = np.empty((128, 2, 256), np.float32)
    mskb[:, 0, :] = np.concatenate([tri, one], axis=1)
    mskb[:, 1, :] = np.concatenate([zero, tri], axis=1)
    return mska, mskb


def build_in_maps(x, Wq, Wk, Wv, Wo):
    bf = ml_dtypes.bfloat16
    f8 = ml_dtypes.float8_e4m3
    segs = _segment_list()
    padded = segs + segs[:N_CORES * SEGS_PER_CORE - len(segs)]
    mska, mskb = _masks()
    wq = (np.asarray(Wq, np.float64) * (EXP_A * SCALE)).astype(bf)
    wk = np.asarray(Wk, np.float64).astype(bf)
    w2 = (np.asarray(Wv, np.float64) @ np.asarray(Wo, np.float64)).astype(bf)
    ones8 = np.ones((C, 2, 32), f8)
    x = np.asarray(x, np.float32)
    in_maps = []
    for core in range(N_CORES):
        xseg = np.empty((SEGS_PER_CORE, C, M), dtype=bf)
        for k in range(SEGS_PER_CORE):
            b, w, r, t = padded[core * SEGS_PER_CORE + k]
            xseg[k] = x[b, t * w + r * np.arange(M), :].T.astype(bf)
        in_maps.append({
            "xseg": xseg,
            "wq": wq, "wk": wk, "w2": w2,
            "mska": mska.astype(f8), "mskb": mskb.astype(f8),
            "mska32": mska, "mskb32": mskb,
            "ones8": ones8,
        })
    return in_maps, padded


def combine(results, padded):
    numer = np.zeros((B, N, C), dtype=np.float64)
    den = np.zeros((B, N), dtype=np.float64)
    seen = set()
    for core in range(N_CORES):
        for k in range(SEGS_PER_CORE):
            key = padded[core * SEGS_PER_CORE + k]
            if key in seen:
                continue
            seen.add(key)
            b, w, r, t = key
            pos = t * w + r * np.arange(M)
            numer[b, pos, :] += results[core]["u"][k].T.astype(np.float64)
            den[b, pos] += results[core]["d"][k].reshape(-1).astype(np.float64)
    return (numer / den[..., None]).astype(np.float32)


def kernel(x, Wq, Wk, Wv, Wo):
    from concourse.bass_utils import run_bass_kernel_spmd

    x = np.asarray(x, dtype=np.float32)
    nc = get_nc()
    in_maps, padded = build_in_maps(x, Wq, Wk, Wv, Wo)
    res = run_bass_kernel_spmd(nc, in_maps, core_ids=list(range(N_CORES)))
    return combine(res.results, padded)


if __name__ == "__main__":
    rng = np.random.default_rng(0)
    x = rng.standard_normal((B, N, C)).astype(np.float32)
    Wq, Wk, Wv, Wo = [
        (rng.standard_normal((C, C)) / np.sqrt(C)).astype(np.float32)
        for _ in range(4)
    ]
    out = kernel(x, Wq, Wk, Wv, Wo)
    print("out", out.shape, out.dtype, np.abs(out).max())
